# revision 1
# baseline (speedup 1.0000x reference)
"""Multi-head attention (B=2, S=4096, D=512, H=8) on 8 Trainium2 NeuronCores.

Sharding: batch x head-pair parallelism. Core c handles batch b = c // 4 and
heads {2*(c%4), 2*(c%4)+1} (128 contiguous rows of the QKV projection
weights, Megatron column-parallel; Wo row-parallel with the partial-sum
reduction done on the host at gather time).

Per-core device program (identical on all cores, different data; matmul
operands in bf16, all accumulation in fp32 PSUM):
  1. Project K, Q: khT/qhT [128(hd), 4096] = W @ x.T via 4 contraction chunks.
     The 1/sqrt(DK) score scale and biases are folded into Wq/bq host-side.
  2. Project V the same way, then PE-transpose to vh [4096(s), 64] per head,
     augmented with a ones column (65th) so the AV matmul also produces the
     softmax denominator.
  3. Attention, per 512-wide query block, per head, per pair of 128-wide key
     tiles (paired so each ACT exp call covers 1024 elements of free dim):
     scoresT [128(k), 512(q)] = kh_tile @ qh_block        (PE)
     expT = exp(scoresT) over both tiles of the pair      (ACT, PSUM->SBUF)
     av [65, 512] += vh_aug_tile.T @ expT                 (PE accumulate;
                                         row 64 = sum_k exp = denominator)
  4. Normalize: recip(denominator) (DVE), broadcast across 64 partitions
     (GpSimd), multiply (DVE) -> avn [64, 512] bf16 per head.
  5. Output projection: out[s,:] += avn_h.T @ WoT_h for both heads (K=64
     accumulating matmuls), DVE-evacuate, DMA to HBM.

Host gathers: out[b] = sum of the 4 per-core partials + bv @ Wo.T + bo.
"""

import ml_dtypes
import numpy as np

import concourse.mybir as mybir
import concourse.tile as tile
from concourse import bacc
from concourse.bass_utils import run_bass_kernel_spmd
from concourse.masks import make_identity

F32 = mybir.dt.float32
BF16 = mybir.dt.bfloat16
EXP = mybir.ActivationFunctionType.Exp
ADD = mybir.AluOpType.add
MULT = mybir.AluOpType.mult
NPBF16 = ml_dtypes.bfloat16

B, S, D, H = 2, 4096, 512, 8
DK = D // H          # 64
HPC = 2              # heads per core
HD = HPC * DK        # 128 head-dims per core
N_CORES = 8
QB = 512             # query block (matmul free dim)
KT = 128             # key tile (partition dim)
NCH = D // 128       # 4 contraction chunks for the projections


def mha_tile_kernel(tc, out_ap, ins, seq=S, dbg=None):
    """Emit the per-core MHA program. `ins` maps name -> DRAM AP."""
    nc = tc.nc
    nqb, nkt = seq // QB, seq // KT

    xq, xk, xv = ins["qt"], ins["kt"], ins["vt"]
    const = tc.alloc_tile_pool(name="const", bufs=1)
    sb = tc.alloc_tile_pool(name="sb", bufs=2)
    pps = tc.alloc_tile_pool(name="pps", bufs=2, space="PSUM")

    # --- constants ---
    wq_sb = const.tile([128, NCH, 128], BF16, tag="wq", name="wq_sb")
    wk_sb = const.tile([128, NCH, 128], BF16, tag="wk", name="wk_sb")
    wv_sb = const.tile([128, NCH, 128], BF16, tag="wv", name="wv_sb")
    for w_sb, name in ((wq_sb, "wq"), (wk_sb, "wk"), (wv_sb, "wv")):
        nc.sync.dma_start(w_sb, ins[name].rearrange("(c p) m -> p c m", p=128))
    wo0_sb = const.tile([64, QB], BF16, tag="wo0", name="wo0_sb")
    wo1_sb = const.tile([64, QB], BF16, tag="wo1", name="wo1_sb")
    nc.sync.dma_start(wo0_sb, ins["wo0"])
    nc.sync.dma_start(wo1_sb, ins["wo1"])
    bq_sb = const.tile([128, 1], F32, tag="bq", name="bq_sb")
    bk_sb = const.tile([128, 1], F32, tag="bk", name="bk_sb")
    nc.sync.dma_start(bq_sb, ins["bq"])
    nc.sync.dma_start(bk_sb, ins["bk"])

    ident = const.tile([128, 128], BF16, tag="ident", name="ident")
    make_identity(nc, ident)
    ones_sb = const.tile([128, 64], F32, tag="ones", name="ones_sb")
    nc.vector.memset(ones_sb, 1.0)

    # --- persistent activations ---
    qhT = const.tile([128, seq], BF16, tag="qhT", name="qhT")
    khT = const.tile([128, seq], BF16, tag="khT", name="khT")
    # vh per head: nkt tiles of [128, 65]; column 64 is the ones column.
    vh = [
        const.tile([128, nkt * 65], BF16, tag=f"vh{h}", name=f"vh{h}")
        for h in range(HPC)
    ]
    for h in range(HPC):
        ones_col = vh[h].rearrange("p (j c) -> p j c", c=65)[:, :, 64]
        nc.vector.tensor_copy(out=ones_col, in_=ones_sb[:, 0:nkt])

    # --- projections of K and Q: dstT[hd, s] = W @ x.T (+ bias) ---
    def project_T(x_dram, w_sb, bias, dstT):
        xc = [
            sb.tile([128, seq], BF16, tag="xchunk", bufs=NCH + 1, name=f"xc{c}")
            for c in range(NCH)
        ]
        for c in range(NCH):
            nc.sync.dma_start(xc[c], x_dram[c * 128 : (c + 1) * 128, :])
        for qb in range(nqb):
            acc = pps.tile([128, QB], F32, tag="proj", bufs=4, name="prj")
            for c in range(NCH):
                nc.tensor.matmul(
                    acc,
                    lhsT=w_sb[:, c, :],
                    rhs=xc[c][:, qb * QB : (qb + 1) * QB],
                    start=(c == 0),
                    stop=(c == NCH - 1),
                )
            dst = dstT[:, qb * QB : (qb + 1) * QB]
            if bias is None:
                nc.vector.tensor_copy(out=dst, in_=acc)
            else:
                nc.vector.tensor_scalar(dst, acc, bias[:, 0:1], None, ADD)

    project_T(xk, wk_sb, bk_sb, khT)
    project_T(xq, wq_sb, bq_sb, qhT)

    # --- V: project to vhT then PE-transpose into vh[s, d] tiles ---
    vhT = sb.tile([128, seq], BF16, tag="vhT", bufs=1, name="vhT")
    project_T(xv, wv_sb, None, vhT)
    for j in range(nkt):
        tp = pps.tile([128, 128], BF16, tag="tp", bufs=2, name="tp")
        nc.tensor.transpose(tp, vhT[:, j * 128 : (j + 1) * 128], ident)
        for h in range(HPC):
            nc.vector.tensor_copy(
                out=vh[h][:, j * 65 : j * 65 + 64],
                in_=tp[:, h * 64 : (h + 1) * 64],
            )
    pps.release()
    ps = tc.alloc_tile_pool(name="ps", bufs=2, space="PSUM")

    if dbg is not None:
        nc.sync.dma_start(dbg["qhT"], qhT)
        nc.sync.dma_start(dbg["khT"], khT)
        nc.sync.dma_start(dbg["vh0"], vh[0])
        nc.sync.dma_start(dbg["vh1"], vh[1])

    # --- attention + output projection, per query block ---
    # Normalize/out-projection is deferred one q-block so the PE queue never
    # waits on the DVE reciprocal (head-of-line stalls re-throttle HAM).
    def norm_and_proj(av_sb, qb):
        avn = []
        for h in range(HPC):
            r_sb = sb.tile([128, QB], F32, tag="r", bufs=2, name="r_sb")
            nc.vector.reciprocal(out=r_sb[64:65, :], in_=av_sb[h][64:65, :])
            bc = ps.tile([64, QB], F32, tag="bc", bufs=1, name="bc")
            nc.tensor.matmul(
                bc,
                lhsT=ones_sb[64:65, :],
                rhs=r_sb[64:65, :],
                start=True,
                stop=True,
            )
            bc_sb = sb.tile([64, QB], F32, tag="bcs", bufs=2, name="bc_sb")
            nc.vector.tensor_copy(out=bc_sb, in_=bc)
            a = sb.tile([64, QB], BF16, tag=f"avn{h}", bufs=2, name=f"avn{h}")
            nc.vector.tensor_tensor(a, av_sb[h][0:64, :], bc_sb, MULT)
            avn.append(a)
            if dbg is not None:
                nc.sync.dma_start(dbg[f"den{h}"][qb : qb + 1, :], av_sb[h][64:65, :])
                nc.sync.dma_start(dbg[f"r{h}"][qb : qb + 1, :], r_sb[64:65, :])
                nc.sync.dma_start(dbg[f"bc{h}"][qb * 64 : (qb + 1) * 64, :], bc_sb)
                nc.sync.dma_start(
                    dbg[f"av{h}"][qb * 65 : (qb + 1) * 65, :], av_sb[h][0:65, :]
                )
        for st in range(QB // 128):
            ssl = slice(st * 128, (st + 1) * 128)
            op = ps.tile([128, QB], F32, tag="op", bufs=1, name="op")
            nc.tensor.matmul(
                op, lhsT=avn[0][:, ssl], rhs=wo0_sb, start=True, stop=False
            )
            nc.tensor.matmul(
                op, lhsT=avn[1][:, ssl], rhs=wo1_sb, start=False, stop=True
            )
            ost = sb.tile([128, QB], F32, tag="ost", bufs=3, name="ost")
            nc.vector.tensor_copy(out=ost, in_=op)
            nc.sync.dma_start(
                out_ap[qb * QB + st * 128 : qb * QB + (st + 1) * 128, :], ost
            )

    KPG = 2  # key tiles per exp group (one ACT call covers KPG*QB elements)
    pending = None
    for qb in range(nqb):
        qsl = slice(qb * QB, (qb + 1) * QB)
        # per-head SBUF accumulators; each key group's AV partial is matmul'd
        # into a short-lived PSUM tile and DVE-added here, so no PSUM tile
        # lives across the q-block boundary and q-blocks fully pipeline
        av_sb = [
            sb.tile([128, QB], F32, tag=f"avsb{h}", bufs=2, name=f"avsb{h}")
            for h in range(HPC)
        ]

        avg_state = [None, None]

        def av_group(ktg, ex, av_sb=av_sb, avg_state=avg_state):
            # PSUM partial spans AVGG consecutive ktg groups before the DVE
            # add, bounding DVE work while keeping PSUM tiles short-lived.
            AVGG = 2
            for h in range(HPC):
                if ktg % AVGG == 0:
                    avg_state[h] = ps.tile(
                        [128, QB], F32, tag="avg", bufs=2, name="avg"
                    )
                avg = avg_state[h]
                for part in range(KPG):
                    kt_i = KPG * ktg + part
                    nc.tensor.matmul(
                        avg[0:65, :],
                        lhsT=vh[h][:, kt_i * 65 : kt_i * 65 + 65],
                        rhs=ex[h][:, part * QB : (part + 1) * QB],
                        start=(ktg % AVGG == 0 and part == 0),
                        stop=(ktg % AVGG == AVGG - 1 and part == KPG - 1),
                    )
                if ktg % AVGG == AVGG - 1:
                    if ktg < AVGG:
                        nc.vector.tensor_copy(
                            out=av_sb[h][0:65, :], in_=avg[0:65, :]
                        )
                    else:
                        nc.vector.tensor_tensor(
                            av_sb[h][0:65, :], avg[0:65, :], av_sb[h][0:65, :], ADD
                        )

        # AV matmuls run one key-group behind the score matmuls so the PE
        # stream never waits on ACT exp (PE idle gaps re-throttle HAM).
        prev_av = None
        for ktg in range(nkt // KPG):
            # both heads' score super-tiles (KPG key tiles each); h0/h1
            # matmuls interleaved so adjacent PE instructions hit disjoint
            # row groups and run concurrently
            sc = [
                ps.tile([128, KPG * QB], F32, tag="sc", bufs=2, name=f"sc{h}")
                for h in range(HPC)
            ]
            for part in range(KPG):
                kt_i = KPG * ktg + part
                for h in range(HPC):
                    hp = slice(h * 64, (h + 1) * 64)
                    nc.tensor.matmul(
                        sc[h][:, part * QB : (part + 1) * QB],
                        lhsT=khT[hp, kt_i * KT : (kt_i + 1) * KT],
                        rhs=qhT[hp, qsl],
                        start=True,
                        stop=True,
                    )
            ex = []
            for h in range(HPC):
                e = sb.tile([128, KPG * QB], BF16, tag="ex", bufs=6, name="ex")
                nc.scalar.activation(e, sc[h], EXP)
                ex.append(e)
            if prev_av is not None:
                av_group(*prev_av)
            prev_av = (ktg, ex)
        av_group(*prev_av)
        if pending is not None:
            norm_and_proj(*pending)
        pending = (av_sb, qb)
    norm_and_proj(*pending)

    ps.release()
    sb.release()
    const.release()


def build_bass(seq=S, debug_outs=False):
    nc = bacc.Bacc(
        "TRN2",
        debug=False,
        enable_asserts=False,
        target_bir_lowering=False,
    )
    ins = {}
    shapes = {
        "qt": (D, seq), "kt": (D, seq), "vt": (D, seq),
        "wq": (D, HD), "wk": (D, HD), "wv": (D, HD),
        "wo0": (64, D), "wo1": (64, D),
        "bq": (HD, 1), "bk": (HD, 1),
    }
    bf16_names = {"qt", "kt", "vt", "wq", "wk", "wv", "wo0", "wo1"}
    for name, shape in shapes.items():
        dt = BF16 if name in bf16_names else F32
        ins[name] = nc.dram_tensor(name, list(shape), dt, kind="ExternalInput").ap()
    out = nc.dram_tensor("out", [seq, D], F32, kind="ExternalOutput").ap()
    dbg = None
    if debug_outs:
        nkt, nqb = seq // KT, seq // QB
        dbg_shapes = {
            "qhT": ((128, seq), BF16), "khT": ((128, seq), BF16),
            "vh0": ((128, nkt * 65), BF16), "vh1": ((128, nkt * 65), BF16),
            "den0": ((nqb, QB), F32), "den1": ((nqb, QB), F32),
            "r0": ((nqb, QB), F32), "r1": ((nqb, QB), F32),
            "bc0": ((nqb * 64, QB), F32), "bc1": ((nqb * 64, QB), F32),
            "av0": ((nqb * 65, QB), F32), "av1": ((nqb * 65, QB), F32),
        }
        dbg = {
            n: nc.dram_tensor(f"dbg_{n}", list(sh), dt, kind="ExternalOutput").ap()
            for n, (sh, dt) in dbg_shapes.items()
        }
    with tile.TileContext(nc) as tc:
        mha_tile_kernel(tc, out, ins, seq=seq, dbg=dbg)
    nc.compile()
    return nc


def shard_inputs(q, k, v, Wq, bq, Wk, bk, Wv, bv, Wo, bo, seq=S):
    """Host-side shard prep. Returns (in_maps, const_vec)."""
    scale = 1.0 / np.sqrt(np.float32(DK))
    q, k, v = (np.asarray(x, np.float32) for x in (q, k, v))
    Wq, bq, Wk, bk, Wv, bv, Wo, bo = (
        np.asarray(x, np.float32) for x in (Wq, bq, Wk, bk, Wv, bv, Wo, bo)
    )
    bf = lambda x: np.ascontiguousarray(x).astype(NPBF16)
    in_maps = []
    for c in range(N_CORES):
        b = c // 4
        rows = slice(128 * (c % 4), 128 * (c % 4) + 128)
        in_maps.append({
            "qt": bf(q[b].T),
            "kt": bf(k[b].T),
            "vt": bf(v[b].T),
            "wq": bf((Wq[rows, :] * scale).T),
            "wk": bf(Wk[rows, :].T),
            "wv": bf(Wv[rows, :].T),
            "wo0": bf(Wo[:, rows][:, 0:64].T),
            "wo1": bf(Wo[:, rows][:, 64:128].T),
            "bq": np.ascontiguousarray((bq[rows] * scale).reshape(HD, 1)),
            "bk": np.ascontiguousarray(bk[rows].reshape(HD, 1)),
        })
    const_vec = (bv @ Wo.T + bo).astype(np.float32)
    return in_maps, const_vec


_NC_CACHE = {}


def run(inputs, seq=S, trace=False, trace_kwargs=None):
    if seq not in _NC_CACHE:
        _NC_CACHE[seq] = build_bass(seq=seq)
    nc = _NC_CACHE[seq]
    in_maps, const_vec = shard_inputs(**inputs, seq=seq)
    res = run_bass_kernel_spmd(
        nc,
        in_maps,
        core_ids=list(range(N_CORES)),
        trace=trace,
        **(trace_kwargs or {}),
    )
    out = np.zeros((B, seq, D), dtype=np.float32)
    for c in range(N_CORES):
        out[c // 4] += res.results[c]["out"]
    out += const_vec[None, None, :]
    return out, res


def kernel(**inputs):
    out, _ = run(inputs)
    return out

